# revision 1
# baseline (speedup 1.0000x reference)
"""MoE kernel for TRN2, 8 NeuronCores, data-parallel over the batch dim.

Reference computation (B=8192, D=1024, H=1024, E=16):
    weights = softmax(x @ Wg + bg, axis=1)            # [B, E]
    h       = relu(einsum('bd,edh->beh', x, W1) + b1) # [B, E, H]
    eo      = einsum('beh,eh->be', h, W2) + b2        # [B, E]
    out     = sum(eo * weights, axis=1, keepdims=True)# [B, 1]

Strategy:
  - Shard B over 8 cores (1024 rows/core); weights replicated.
  - All matmuls contract over the partition dim, so x is transposed on the
    HOST (free) and each core gets xT [D, 1024] resident in SBUF.
  - Stage 1 (per (e, h_tile)=t of 128): psum[h=128, b=512x2] accumulated
    over 8 d-tiles; float32r matmuls (full PE rate at N>=256).
  - ReLU+b1 via ScalarE activation (bias is per-partition in h-major layout).
  - Stage 2: W2 built block-diagonal on host -> every t contributes one
    [128hx16e] @ [128h, 512b] matmul accumulating into ONE [16, 1024] psum
    tile; all 16 experts' outputs land stacked on partitions 0..15.
  - Gating: fp32 matmuls into [128b, 16e] psum; softmax along the free dim;
    exp(bg) folded in multiplicatively (softmax is shift/scale invariant).
  - Combine: eoT + b2 -> PE-transpose 16x128 chunks -> [128b, 16e]; multiply
    by gate weights, reduce along free dim -> [128, 1] -> DMA out.
"""

import numpy as np

import concourse.bacc as bacc
import concourse.bass as bass
import concourse.mybir as mybir
from concourse import tile
from concourse.bass_utils import run_bass_kernel_spmd

B, D, H, E = 8192, 1024, 1024, 16
N_CORES = 8
BS = B // N_CORES  # 1024 batch rows per core
NB = BS // 128     # 8 b-tiles of 128
BH = 512           # half-batch moving-operand width (one psum bank)
DT = D // 128      # 8 d-tiles
HT = H // 128      # 8 h-tiles
T = E * HT         # 128 (e, h_tile) pairs

F32 = mybir.dt.float32
F32R = mybir.dt.float32r
AF = mybir.ActivationFunctionType
AX = mybir.AxisListType


def build_bass():
    nc = bacc.Bacc("TRN2", target_bir_lowering=False, debug=False)
    xt_d = nc.dram_tensor("xt", [D, BS], F32R, kind="ExternalInput")
    w1_d = nc.dram_tensor("w1p", [T, 128, DT * 128], F32R, kind="ExternalInput")
    b1t_d = nc.dram_tensor("b1t", [128, T], F32, kind="ExternalInput")
    w2bd_d = nc.dram_tensor("w2bd", [128, T * E], F32R, kind="ExternalInput")
    wgp_d = nc.dram_tensor("wgp", [128, DT * E], F32R, kind="ExternalInput")
    ebg_d = nc.dram_tensor("ebg", [128, E], F32, kind="ExternalInput")
    b2_d = nc.dram_tensor("b2p", [E, 1], F32, kind="ExternalInput")
    id16_d = nc.dram_tensor("id16", [E, E], F32, kind="ExternalInput")
    y_d = nc.dram_tensor("y", [BS, 1], F32, kind="ExternalOutput")

    with tile.TileContext(nc) as tc:
        with (
            tc.tile_pool(name="const", bufs=1) as cpool,
            tc.tile_pool(name="w1", bufs=4) as w1pool,
            tc.tile_pool(name="hrelu", bufs=4) as hpool,
            tc.tile_pool(name="sm", bufs=2) as smpool,
            tc.tile_pool(name="ps_h", bufs=2, space=bass.MemorySpace.PSUM) as psh,
            tc.tile_pool(name="ps_eo", bufs=1, space=bass.MemorySpace.PSUM) as pseo,
            tc.tile_pool(name="ps_s", bufs=2, space=bass.MemorySpace.PSUM) as pss,
        ):
            # ---- resident tensors ----
            xt_sb = []
            for d in range(DT):
                tl = cpool.tile([128, BS], F32R, tag=f"xt{d}")
                nc.sync.dma_start(tl[:], xt_d[d * 128:(d + 1) * 128, :])
                xt_sb.append(tl)
            w2bd_sb = cpool.tile([128, T * E], F32R, tag="w2bd")
            nc.sync.dma_start(w2bd_sb[:], w2bd_d[:])
            b1t_sb = cpool.tile([128, T], F32, tag="b1t")
            nc.sync.dma_start(b1t_sb[:], b1t_d[:])
            wgp_sb = cpool.tile([128, DT * E], F32R, tag="wgp")
            nc.sync.dma_start(wgp_sb[:], wgp_d[:])
            ebg_sb = cpool.tile([128, E], F32, tag="ebg")
            nc.sync.dma_start(ebg_sb[:], ebg_d[:])
            b2_sb = cpool.tile([E, 1], F32, tag="b2")
            nc.sync.dma_start(b2_sb[:], b2_d[:])
            id16_sb = cpool.tile([E, E], F32, tag="id16")
            nc.sync.dma_start(id16_sb[:], id16_d[:])
            w_all = cpool.tile([128, NB, E], F32, tag="wall")  # gate weights
            eo_sb = cpool.tile([E, BS], F32, tag="eo")         # expert outs ^T

            # ---- gating: logits -> softmax along free dim ----
            for bt in range(NB):
                ps_g = pss.tile([128, E], F32, tag="sps")
                for d in range(DT):
                    nc.tensor.matmul(
                        ps_g[:],
                        xt_sb[d][:, bt * 128:(bt + 1) * 128],
                        wgp_sb[:, d * E:(d + 1) * E],
                        start=(d == 0), stop=(d == DT - 1),
                        skip_group_check=True,
                    )
                pexp = smpool.tile([128, E], F32, tag="pexp")
                nc.scalar.activation(pexp[:], ps_g[:], AF.Exp)
                nc.vector.tensor_mul(pexp[:], pexp[:], ebg_sb[:])
                ssum = smpool.tile([128, 1], F32, tag="ssum")
                nc.vector.reduce_sum(ssum[:], pexp[:], axis=AX.X)
                rsum = smpool.tile([128, 1], F32, tag="rsum")
                nc.vector.reciprocal(rsum[:], ssum[:])
                nc.vector.tensor_scalar_mul(w_all[:, bt, :], pexp[:], rsum[:])

            # ---- main loop over t=(e, h_tile) ----
            eo_ps = pseo.tile([E, BS], F32)

            def emit_stage2(t, hr):
                for bh in range(2):
                    nc.tensor.matmul(
                        eo_ps[:, bh * BH:(bh + 1) * BH],
                        w2bd_sb[:, t * E:(t + 1) * E],
                        hr[:, bh * BH:(bh + 1) * BH],
                        start=(t == 0), stop=(t == T - 1),
                        skip_group_check=True,
                    )

            pending = []
            for t in range(T):
                w1t = w1pool.tile([128, DT * 128], F32R, tag="w1t")
                nc.sync.dma_start(w1t[:], w1_d[t, :, :])
                ps1 = psh.tile([128, BS], F32, tag="ps1")
                for d in range(DT):
                    lhs = w1t[:, d * 128:(d + 1) * 128]
                    for bh in range(2):
                        nc.tensor.matmul(
                            ps1[:, bh * BH:(bh + 1) * BH],
                            lhs,
                            xt_sb[d][:, bh * BH:(bh + 1) * BH],
                            start=(d == 0), stop=(d == DT - 1),
                            skip_group_check=True,
                        )
                if pending:
                    emit_stage2(*pending.pop())
                hr = hpool.tile([128, BS], F32R, tag="hr")
                for bh in range(2):
                    nc.scalar.activation(
                        hr[:, bh * BH:(bh + 1) * BH],
                        ps1[:, bh * BH:(bh + 1) * BH],
                        AF.Relu,
                        bias=b1t_sb[:, t:t + 1],
                    )
                pending.append((t, hr))
            emit_stage2(*pending.pop())

            # ---- combine: (eoT + b2) -> transpose -> * gates -> reduce ----
            nc.vector.tensor_scalar_add(eo_sb[:], eo_ps[:], b2_sb[:])
            for bt in range(NB):
                tps = pss.tile([128, E], F32, tag="sps")
                nc.tensor.transpose(
                    tps[:], eo_sb[:, bt * 128:(bt + 1) * 128], id16_sb[:]
                )
                eo_bt = smpool.tile([128, E], F32, tag="eobt")
                nc.vector.tensor_copy(eo_bt[:], tps[:])
                prod = smpool.tile([128, E], F32, tag="prod")
                nc.vector.tensor_mul(prod[:], eo_bt[:], w_all[:, bt, :])
                y_t = smpool.tile([128, 1], F32, tag="yt")
                nc.vector.reduce_sum(y_t[:], prod[:], axis=AX.X)
                nc.sync.dma_start(y_d[bt * 128:(bt + 1) * 128, :], y_t[:])
    nc.compile()
    return nc


def round_fp32r(a):
    """Round fp32 to the FP32R format: 11-bit mantissa, RNE, low 12 bits 0."""
    u = np.ascontiguousarray(a, dtype=np.float32).view(np.uint32)
    lsb = (u >> np.uint32(12)) & np.uint32(1)
    r = (u + np.uint32(0x7FF) + lsb) & np.uint32(0xFFFFF000)
    return r.view(np.float32)


def prep_inputs(x, W1, b1, W2, b2, Wg, bg):
    """Host-side data prep. Returns (shared_map, per_core_xt)."""
    f = np.float32
    # W1 [E, D, H] -> [t=(e,ht), d_in, (d_t, h_in)] so each t is one
    # contiguous 512KB block whose SBUF layout is [128 d_in, 8 d_t * 128 h]
    w1p = np.ascontiguousarray(
        W1.reshape(E, DT, 128, HT, 128).transpose(0, 3, 2, 1, 4)
        .reshape(T, 128, DT * 128).astype(f))
    w1p = round_fp32r(w1p)
    b1t = np.ascontiguousarray(
        b1.reshape(E, HT, 128).transpose(2, 0, 1).reshape(128, T).astype(f))
    w2bd = np.zeros((128, T, E), dtype=f)
    for t in range(T):
        e, ht = divmod(t, HT)
        w2bd[:, t, e] = W2[e, ht * 128:(ht + 1) * 128]
    w2bd = round_fp32r(w2bd.reshape(128, T * E))
    wgp = np.ascontiguousarray(
        Wg.reshape(DT, 128, E).transpose(1, 0, 2).reshape(128, DT * E).astype(f))
    wgp = round_fp32r(wgp)
    ebg = np.broadcast_to(np.exp(bg.astype(f))[None, :], (128, E)).copy()
    b2p = np.ascontiguousarray(b2.astype(f).reshape(E, 1))
    id16 = np.eye(E, dtype=f)
    shared = {"w1p": w1p, "b1t": b1t, "w2bd": w2bd, "wgp": wgp,
              "ebg": ebg, "b2p": b2p, "id16": id16}
    xT = round_fp32r(np.ascontiguousarray(x.astype(f).T))  # [D, B]
    xts = [np.ascontiguousarray(xT[:, c * BS:(c + 1) * BS]) for c in range(N_CORES)]
    return shared, xts


def run(inputs, trace=False):
    nc = build_bass()
    shared, xts = prep_inputs(**inputs)
    in_maps = [dict(shared, xt=xts[c]) for c in range(N_CORES)]
    res = run_bass_kernel_spmd(
        nc, in_maps, core_ids=list(range(N_CORES)), trace=trace
    )
    y = np.concatenate([r["y"] for r in res.results], axis=0)
    return y, res


def kernel(**inputs):
    y, _ = run(inputs, trace=False)
    return y


if __name__ == "__main__":
    rng = np.random.default_rng(0)
    ins = {
        "x": rng.standard_normal((B, D), dtype=np.float32),
        "W1": rng.standard_normal((E, D, H), dtype=np.float32) / 32,
        "b1": rng.standard_normal((E, H), dtype=np.float32) / 32,
        "W2": rng.standard_normal((E, H), dtype=np.float32) / 32,
        "b2": rng.standard_normal((E,), dtype=np.float32) / 32,
        "Wg": rng.standard_normal((D, E), dtype=np.float32) / 32,
        "bg": rng.standard_normal((E,), dtype=np.float32) / 32,
    }
    y = kernel(**ins)
    print("ok", y.shape, y.dtype)



# revision 3
# speedup vs baseline: 1.2474x; 1.2474x over previous
"""MoE kernel for TRN2, 8 NeuronCores, data-parallel over the batch dim.

Reference computation (B=8192, D=1024, H=1024, E=16):
    weights = softmax(x @ Wg + bg, axis=1)            # [B, E]
    h       = relu(einsum('bd,edh->beh', x, W1) + b1) # [B, E, H]
    eo      = einsum('beh,eh->be', h, W2) + b2        # [B, E]
    out     = sum(eo * weights, axis=1, keepdims=True)# [B, 1]

Strategy (v2):
  - Shard B over 8 cores (1024 rows/core); weights replicated.
  - Everything in bf16 on the PE (tolerance 2e-2 vs ~3e-3 bf16 noise):
    halves W1 DMA traffic and enables fast weight load (FWL).
  - Stage 1 per t=(ht,e): psum[h=128, b=512x2] accumulated over 8 d-tiles.
  - ReLU+b1 on ScalarE -> hr bf16.
  - Stage 2 uses PE column tiling: experts are assigned to the 4 32-col
    groups (e -> group e%4, row e//4), so 4 consecutive t's stage-2
    matmuls run CONCURRENTLY in disjoint col groups (~4x faster than a
    serialized block-diagonal form). Emitted in bursts of 4 t's, one t
    delayed so ReLU has time to land.
  - Gating: stationary Wg produces logits directly as [16e, B] (expert
    e at partition 32*(e%4)+e//4); U = exp(logits + bg) UNNORMALIZED.
    den = sum_e U and num2 = sum_e U*b2 via tiny partition-sum matmuls.
  - Combine: V = U * eo (DVE) -> num1 via 8 partition-sum matmuls ->
    y = (num1 + num2) / den -> single [128, 8] DMA out.
"""

import ml_dtypes
import numpy as np

import concourse.bacc as bacc
import concourse.bass as bass
import concourse.mybir as mybir
from concourse import tile
from concourse.bass_utils import run_bass_kernel_spmd

B, D, H, E = 8192, 1024, 1024, 16
N_CORES = 8
BS = B // N_CORES  # 1024 batch rows per core
BH = 512           # half-batch moving-operand width (one psum bank)
DT = D // 128      # 8 d-tiles
HT = H // 128      # 8 h-tiles
T = E * HT         # 128 tiles; t = ht*16 + e  (e minor)
NCH = BS // 128    # 8 b-chunks of 128

F32 = mybir.dt.float32
BF16 = mybir.dt.bfloat16
AF = mybir.ActivationFunctionType
BF_NP = ml_dtypes.bfloat16


def build_bass():
    nc = bacc.Bacc("TRN2", target_bir_lowering=False, debug=False)
    xt_d = nc.dram_tensor("xt", [128, DT * BS], BF16, kind="ExternalInput")
    w1_d = nc.dram_tensor("w1p", [T, 128, DT * 128], BF16, kind="ExternalInput")
    b1t_d = nc.dram_tensor("b1t", [128, T], F32, kind="ExternalInput")
    w2c_d = nc.dram_tensor("w2c", [128, T * 32], BF16, kind="ExternalInput")
    wgp_d = nc.dram_tensor("wgp4", [128, DT * 128], BF16, kind="ExternalInput")
    bg4_d = nc.dram_tensor("bg4", [128, 1], F32, kind="ExternalInput")
    ob2_d = nc.dram_tensor("ob2", [128, 2], BF16, kind="ExternalInput")
    y_d = nc.dram_tensor("y", [128, NCH], F32, kind="ExternalOutput")

    with tile.TileContext(nc) as tc:
        with (
            tc.tile_pool(name="const", bufs=1) as cpool,
            tc.tile_pool(name="w1", bufs=8) as w1pool,
            tc.tile_pool(name="hrelu", bufs=8) as hpool,
            tc.tile_pool(name="sm", bufs=2) as smpool,
            tc.tile_pool(name="ps_h", bufs=2, space=bass.MemorySpace.PSUM) as psh,
            tc.tile_pool(name="ps_eo", bufs=1, space=bass.MemorySpace.PSUM) as pseo,
            tc.tile_pool(name="ps_s", bufs=1, space=bass.MemorySpace.PSUM) as pss,
        ):
            # ---- resident tensors; gating-critical loads first on the
            # scalar (ACT) HWDGE ring, W1 stream on the sync (SP) ring ----
            wgp_sb = cpool.tile([128, DT * 128], BF16, tag="wgp")
            nc.scalar.dma_start(wgp_sb[:], wgp_d[:])
            bg4_sb = cpool.tile([128, 1], F32, tag="bg4")
            nc.scalar.dma_start(bg4_sb[:], bg4_d[:])
            ob2_sb = cpool.tile([128, 2], BF16, tag="ob2")
            nc.scalar.dma_start(ob2_sb[:], ob2_d[:])
            xt_sb = cpool.tile([128, DT * BS], BF16, tag="xt")
            for dd in range(DT):
                nc.scalar.dma_start(
                    xt_sb[:, dd * BS:(dd + 1) * BS], xt_d[:, dd * BS:(dd + 1) * BS]
                )
            b1t_sb = cpool.tile([128, T], F32, tag="b1t")
            nc.scalar.dma_start(b1t_sb[:], b1t_d[:])
            w2c_sb = cpool.tile([128, T * 32], BF16, tag="w2c")
            nc.scalar.dma_start(w2c_sb[:], w2c_d[:])

            # ---- gating: logits -> [16e rows spread over col groups, B] ----
            gps = psh.tile([128, BS], F32, tag="ps1")
            for dd in range(DT):
                for bh in range(2):
                    nc.tensor.matmul(
                        gps[:, bh * BH:(bh + 1) * BH],
                        wgp_sb[:, dd * 128:(dd + 1) * 128],
                        xt_sb[:, dd * BS + bh * BH: dd * BS + (bh + 1) * BH],
                        start=(dd == 0), stop=(dd == DT - 1),
                        skip_group_check=True,
                    )
            u4 = cpool.tile([128, BS], BF16, tag="u4")
            nc.scalar.activation(u4[:], gps[:], AF.Exp, bias=bg4_sb[:])
            u4f = cpool.tile([128, BS], F32, tag="u4f")
            nc.vector.tensor_copy(u4f[:], u4[:])

            # ---- main loop over t = ht*16 + e ----
            eo_ps = pseo.tile([128, BS], F32)
            pending = []  # [(t, hr), ...] up to 4

            def emit_stage2_burst():
                for bh in range(2):
                    for (tt, hh) in pending:
                        g = (tt % E) % 4
                        nc.tensor.matmul(
                            eo_ps[32 * g:32 * g + 32, bh * BH:(bh + 1) * BH],
                            w2c_sb[:, tt * 32:(tt + 1) * 32],
                            hh[:, bh * BH:(bh + 1) * BH],
                            start=(tt < 4), stop=(tt >= T - 4),
                            skip_group_check=True,
                            tile_position=(0, 32 * g),
                        )
                pending.clear()

            den_ps = pss.tile([128, NCH], F32, tag="sps")
            nb2_ps = pss.tile([128, NCH], F32, tag="sps2")
            rden = cpool.tile([128, NCH], F32, tag="rden")
            num2 = cpool.tile([128, NCH], F32, tag="num2")

            for t in range(T):
                w1t = w1pool.tile([128, DT * 128], BF16, tag="w1t")
                nc.sync.dma_start(w1t[:], w1_d[t, :, :])
                ps1 = psh.tile([128, BS], F32, tag="ps1")
                for dd in range(DT):
                    lhs = w1t[:, dd * 128:(dd + 1) * 128]
                    for bh in range(2):
                        nc.tensor.matmul(
                            ps1[:, bh * BH:(bh + 1) * BH],
                            lhs,
                            xt_sb[:, dd * BS + bh * BH: dd * BS + (bh + 1) * BH],
                            start=(dd == 0), stop=(dd == DT - 1),
                            skip_group_check=True,
                        )
                if len(pending) == 4:
                    emit_stage2_burst()
                if t == 1:
                    # den/num2 partition-sums; u4 is ready by now, PE is warm
                    for j in range(NCH):
                        nc.tensor.matmul(
                            den_ps[:, j:j + 1],
                            u4[:, j * 128:(j + 1) * 128],
                            ob2_sb[:, 0:1],
                            start=True, stop=True, skip_group_check=True,
                        )
                        nc.tensor.matmul(
                            nb2_ps[:, j:j + 1],
                            u4[:, j * 128:(j + 1) * 128],
                            ob2_sb[:, 1:2],
                            start=True, stop=True, skip_group_check=True,
                        )
                if t == 2:
                    nc.vector.reciprocal(rden[:], den_ps[:])
                    nc.vector.tensor_copy(num2[:], nb2_ps[:])
                hr = hpool.tile([128, BS], BF16, tag="hr")
                for bh in range(2):
                    nc.scalar.activation(
                        hr[:, bh * BH:(bh + 1) * BH],
                        ps1[:, bh * BH:(bh + 1) * BH],
                        AF.Relu,
                        bias=b1t_sb[:, t:t + 1],
                    )
                pending.append((t, hr))
            emit_stage2_burst()

            # ---- combine: V = U*eo -> num1 -> y = (num1+num2)*rden ----
            v4 = cpool.tile([128, BS], BF16, tag="v4")
            nc.vector.tensor_mul(v4[:], eo_ps[:], u4f[:])
            num_ps = pss.tile([128, NCH], F32, tag="sps")
            for j in range(NCH):
                nc.tensor.matmul(
                    num_ps[:, j:j + 1],
                    v4[:, j * 128:(j + 1) * 128],
                    ob2_sb[:, 0:1],
                    start=True, stop=True, skip_group_check=True,
                )
            ysb = smpool.tile([128, NCH], F32, tag="ysb")
            nc.vector.tensor_add(ysb[:], num_ps[:], num2[:])
            nc.vector.tensor_mul(ysb[:], ysb[:], rden[:])
            nc.sync.dma_start(y_d[:], ysb[:])
    nc.compile()
    return nc


def prep_inputs(x, W1, b1, W2, b2, Wg, bg):
    """Host-side data prep. Returns (shared_map, per_core_xt)."""
    f = np.float32
    # W1 [E, D, H] -> [t=(ht,e), d_in, (d_t, h_in)]
    w1p = np.ascontiguousarray(
        W1.reshape(E, DT, 128, HT, 128).transpose(3, 0, 2, 1, 4)
        .reshape(T, 128, DT * 128)).astype(BF_NP)
    b1t = np.ascontiguousarray(
        b1.reshape(E, HT, 128).transpose(2, 1, 0).reshape(128, T).astype(f))
    # stage-2 stationaries: expert e -> col group g=e%4, row k=e//4
    w2c = np.zeros((128, T, 32), dtype=f)
    for t in range(T):
        ht, e = divmod(t, E)
        k, g = divmod(e, 4)
        w2c[:, t, k] = W2[e, ht * 128:(ht + 1) * 128]
    w2c = w2c.reshape(128, T * 32).astype(BF_NP)
    # gating stationary: col 32g+k = Wg[:, 4k+g], rest zero
    wgp4 = np.zeros((DT, 128, 128), dtype=f)
    bg4 = np.full((128, 1), -30.0, dtype=f)
    ob2 = np.zeros((128, 2), dtype=f)
    ob2[:, 0] = 1.0
    for e in range(E):
        k, g = divmod(e, 4)
        wgp4[:, :, 32 * g + k] = Wg[:, e].reshape(DT, 128)
        bg4[32 * g + k, 0] = bg[e]
        ob2[32 * g + k, 1] = b2[e]
    wgp4 = np.ascontiguousarray(
        wgp4.transpose(1, 0, 2).reshape(128, DT * 128)).astype(BF_NP)
    ob2 = ob2.astype(BF_NP)
    shared = {"w1p": w1p, "b1t": b1t, "w2c": w2c, "wgp4": wgp4,
              "bg4": bg4, "ob2": ob2}
    xT = np.ascontiguousarray(np.asarray(x, dtype=f).T)  # [D, B]
    xts = []
    for c in range(N_CORES):
        xc = xT[:, c * BS:(c + 1) * BS]  # [D, BS]
        xc = np.ascontiguousarray(
            xc.reshape(DT, 128, BS).transpose(1, 0, 2).reshape(128, DT * BS))
        xts.append(xc.astype(BF_NP))
    return shared, xts


def run(inputs, trace=False):
    nc = build_bass()
    shared, xts = prep_inputs(**inputs)
    in_maps = [dict(shared, xt=xts[c]) for c in range(N_CORES)]
    res = run_bass_kernel_spmd(
        nc, in_maps, core_ids=list(range(N_CORES)), trace=trace
    )
    # y dram is [128, NCH] with y[p, j] = out[j*128 + p]
    y = np.concatenate(
        [np.asarray(r["y"], dtype=np.float32).T.reshape(BS, 1)
         for r in res.results], axis=0)
    return y, res


def kernel(**inputs):
    y, _ = run(inputs, trace=False)
    return y


if __name__ == "__main__":
    rng = np.random.default_rng(0)
    ins = {
        "x": rng.standard_normal((B, D), dtype=np.float32),
        "W1": rng.standard_normal((E, D, H), dtype=np.float32) / 32,
        "b1": rng.standard_normal((E, H), dtype=np.float32) / 32,
        "W2": rng.standard_normal((E, H), dtype=np.float32) / 32,
        "b2": rng.standard_normal((E,), dtype=np.float32) / 32,
        "Wg": rng.standard_normal((D, E), dtype=np.float32) / 32,
        "bg": rng.standard_normal((E,), dtype=np.float32) / 32,
    }
    y = kernel(**ins)
    print("ok", y.shape, y.dtype)


# revision 9
# speedup vs baseline: 1.2673x; 1.0159x over previous
"""MoE kernel for TRN2, 8 NeuronCores, data-parallel over the batch dim.

Reference computation (B=8192, D=1024, H=1024, E=16):
    weights = softmax(x @ Wg + bg, axis=1)            # [B, E]
    h       = relu(einsum('bd,edh->beh', x, W1) + b1) # [B, E, H]
    eo      = einsum('beh,eh->be', h, W2) + b2        # [B, E]
    out     = sum(eo * weights, axis=1, keepdims=True)# [B, 1]

Strategy (v2):
  - Shard B over 8 cores (1024 rows/core); weights replicated.
  - Everything in bf16 on the PE (tolerance 2e-2 vs ~3e-3 bf16 noise):
    halves W1 DMA traffic and enables fast weight load (FWL).
  - Stage 1 per t=(ht,e): psum[h=128, b=512x2] accumulated over 8 d-tiles.
  - ReLU+b1 on ScalarE -> hr bf16.
  - Stage 2 uses PE column tiling: experts are assigned to the 4 32-col
    groups (e -> group e%4, row e//4), so 4 consecutive t's stage-2
    matmuls run CONCURRENTLY in disjoint col groups (~4x faster than a
    serialized block-diagonal form). Emitted in bursts of 4 t's, one t
    delayed so ReLU has time to land.
  - Gating: stationary Wg produces logits directly as [16e, B] (expert
    e at partition 32*(e%4)+e//4); U = exp(logits + bg) UNNORMALIZED.
    den = sum_e U and num2 = sum_e U*b2 via tiny partition-sum matmuls.
  - Combine: V = U * eo (DVE) -> num1 via 8 partition-sum matmuls ->
    y = (num1 + num2) / den -> single [128, 8] DMA out.
"""

import ml_dtypes
import numpy as np

import concourse.bacc as bacc
import concourse.bass as bass
import concourse.mybir as mybir
from concourse import tile
from concourse.bass_utils import run_bass_kernel_spmd

B, D, H, E = 8192, 1024, 1024, 16
N_CORES = 8
BS = B // N_CORES  # 1024 batch rows per core
BH = 512           # half-batch moving-operand width (one psum bank)
DT = D // 128      # 8 d-tiles
HT = H // 128      # 8 h-tiles
T = E * HT         # 128 tiles; t = ht*16 + e  (e minor)
NCH = BS // 128    # 8 b-chunks of 128

F32 = mybir.dt.float32
BF16 = mybir.dt.bfloat16
AF = mybir.ActivationFunctionType
BF_NP = ml_dtypes.bfloat16


def build_bass():
    nc = bacc.Bacc("TRN2", target_bir_lowering=False, debug=False)
    xt_d = nc.dram_tensor("xt", [128, DT * BS], BF16, kind="ExternalInput")
    w1_d = nc.dram_tensor("w1p", [T, 128, DT * 128], BF16, kind="ExternalInput")
    b1t_d = nc.dram_tensor("b1t", [128, T], F32, kind="ExternalInput")
    w2c_d = nc.dram_tensor("w2c", [128, T * 32], BF16, kind="ExternalInput")
    wgp_d = nc.dram_tensor("wgp4", [128, DT * 128], BF16, kind="ExternalInput")
    bg4_d = nc.dram_tensor("bg4", [128, 1], F32, kind="ExternalInput")
    ob2_d = nc.dram_tensor("ob2", [128, 2], BF16, kind="ExternalInput")
    y_d = nc.dram_tensor("y", [128, NCH], F32, kind="ExternalOutput")

    with tile.TileContext(nc) as tc:
        with (
            tc.tile_pool(name="const", bufs=1) as cpool,
            tc.tile_pool(name="w1", bufs=8) as w1pool,
            tc.tile_pool(name="hrelu", bufs=12) as hpool,
            tc.tile_pool(name="sm", bufs=2) as smpool,
            tc.tile_pool(name="ps_h", bufs=2, space=bass.MemorySpace.PSUM) as psh,
            tc.tile_pool(name="ps_eo", bufs=1, space=bass.MemorySpace.PSUM) as pseo,
            tc.tile_pool(name="ps_s", bufs=1, space=bass.MemorySpace.PSUM) as pss,
        ):
            # ---- resident tensors; xt is split across BOTH HWDGE rings
            # (scalar=ACT gets d0..3, sync=SP gets d4..7 ahead of the W1
            # stream) so gating can start ~2x sooner ----
            wgp_sb = cpool.tile([128, DT * 128], BF16, tag="wgp")
            nc.scalar.dma_start(wgp_sb[:], wgp_d[:])
            bg4_sb = cpool.tile([128, 1], F32, tag="bg4")
            nc.scalar.dma_start(bg4_sb[:], bg4_d[:])
            ob2_sb = cpool.tile([128, 2], BF16, tag="ob2")
            nc.scalar.dma_start(ob2_sb[:], ob2_d[:])
            b1t_sb = cpool.tile([128, T], F32, tag="b1t")
            nc.scalar.dma_start(b1t_sb[:], b1t_d[:])
            xt_sb = cpool.tile([128, DT * BS], BF16, tag="xt")
            for dd in range(DT):
                eng = nc.scalar if dd < 4 else nc.sync
                eng.dma_start(
                    xt_sb[:, dd * BS:(dd + 1) * BS], xt_d[:, dd * BS:(dd + 1) * BS]
                )
            w2c_sb = cpool.tile([128, T * 32], BF16, tag="w2c")
            nc.scalar.dma_start(w2c_sb[:], w2c_d[:])

            # ---- gating: logits -> [16e rows spread over col groups, B];
            # d-order interleaved to match the two xt DMA streams ----
            gps = psh.tile([128, BS], F32, tag="ps1")
            gate_order = [0, 4, 1, 5, 2, 6, 3, 7]
            for i, dd in enumerate(gate_order):
                for bh in range(2):
                    nc.tensor.matmul(
                        gps[:, bh * BH:(bh + 1) * BH],
                        wgp_sb[:, dd * 128:(dd + 1) * 128],
                        xt_sb[:, dd * BS + bh * BH: dd * BS + (bh + 1) * BH],
                        start=(i == 0), stop=(i == DT - 1),
                        skip_group_check=True,
                    )
            u4 = cpool.tile([128, BS], BF16, tag="u4")
            nc.scalar.activation(u4[:], gps[:], AF.Exp, bias=bg4_sb[:])
            u4f = cpool.tile([128, BS], F32, tag="u4f")
            nc.vector.tensor_copy(u4f[:], u4[:])

            # ---- main loop over t = ht*16 + e ----
            eo_ps = pseo.tile([128, BS], F32)
            pending = []  # [(t, hr), ...] up to BURST
            BURST = 8

            def emit_stage2_burst():
                # consecutive t's alternate col groups, so MMs overlap
                for bh in range(2):
                    for (tt, hh) in pending:
                        g = (tt % E) % 4
                        nc.tensor.matmul(
                            eo_ps[32 * g:32 * g + 32, bh * BH:(bh + 1) * BH],
                            w2c_sb[:, tt * 32:(tt + 1) * 32],
                            hh[:, bh * BH:(bh + 1) * BH],
                            start=(tt < 4), stop=(tt >= T - 4),
                            skip_group_check=True,
                            tile_position=(0, 32 * g),
                        )
                pending.clear()

            den_ps = pss.tile([128, NCH], F32, tag="sps")
            nb2_ps = pss.tile([128, NCH], F32, tag="sps2")
            rden = cpool.tile([128, NCH], F32, tag="rden")
            num2 = cpool.tile([128, NCH], F32, tag="num2")

            for t in range(T):
                w1t = w1pool.tile([128, DT * 128], BF16, tag="w1t")
                nc.sync.dma_start(w1t[:], w1_d[t, :, :])
                ps1 = psh.tile([128, BS], F32, tag="ps1")
                for dd in range(DT):
                    lhs = w1t[:, dd * 128:(dd + 1) * 128]
                    for bh in range(2):
                        nc.tensor.matmul(
                            ps1[:, bh * BH:(bh + 1) * BH],
                            lhs,
                            xt_sb[:, dd * BS + bh * BH: dd * BS + (bh + 1) * BH],
                            start=(dd == 0), stop=(dd == DT - 1),
                            skip_group_check=True,
                        )
                if len(pending) == BURST or t == T - 1:
                    # at t == T-1 flush t'=120..126 now so only t=127's two
                    # matmuls have to wait for the final ReLU
                    emit_stage2_burst()
                if t == 1:
                    # den/num2 partition-sums; u4 is ready by now, PE is warm
                    for j in range(NCH):
                        nc.tensor.matmul(
                            den_ps[:, j:j + 1],
                            u4[:, j * 128:(j + 1) * 128],
                            ob2_sb[:, 0:1],
                            start=True, stop=True, skip_group_check=True,
                        )
                        nc.tensor.matmul(
                            nb2_ps[:, j:j + 1],
                            u4[:, j * 128:(j + 1) * 128],
                            ob2_sb[:, 1:2],
                            start=True, stop=True, skip_group_check=True,
                        )
                if t == 2:
                    nc.vector.reciprocal(rden[:], den_ps[:])
                    nc.vector.tensor_copy(num2[:], nb2_ps[:])
                hr = hpool.tile([128, BS], BF16, tag="hr")
                for bh in range(2):
                    nc.scalar.activation(
                        hr[:, bh * BH:(bh + 1) * BH],
                        ps1[:, bh * BH:(bh + 1) * BH],
                        AF.Relu,
                        bias=b1t_sb[:, t:t + 1],
                    )
                pending.append((t, hr))
            emit_stage2_burst()

            # ---- combine: V = U*eo -> num1 -> y = (num1+num2)*rden ----
            # V-mul split in halves so the partition-sum matmuls for the
            # first half overlap the DVE on the second half
            v4 = cpool.tile([128, BS], BF16, tag="v4")
            num_ps = pss.tile([128, NCH], F32, tag="sps")
            for half in range(2):
                cols = slice(half * BH, (half + 1) * BH)
                nc.vector.tensor_mul(v4[:, cols], eo_ps[:, cols], u4f[:, cols])
                for j in range(half * 4, half * 4 + 4):
                    nc.tensor.matmul(
                        num_ps[:, j:j + 1],
                        v4[:, j * 128:(j + 1) * 128],
                        ob2_sb[:, 0:1],
                        start=True, stop=True, skip_group_check=True,
                    )
            ysb = smpool.tile([128, NCH], F32, tag="ysb")
            nc.vector.tensor_add(ysb[:], num_ps[:], num2[:])
            nc.vector.tensor_mul(ysb[:], ysb[:], rden[:])
            nc.sync.dma_start(y_d[:], ysb[:])
    nc.compile()
    return nc


def prep_inputs(x, W1, b1, W2, b2, Wg, bg):
    """Host-side data prep. Returns (shared_map, per_core_xt)."""
    f = np.float32
    # W1 [E, D, H] -> [t=(ht,e), d_in, (d_t, h_in)]
    w1p = np.ascontiguousarray(
        W1.reshape(E, DT, 128, HT, 128).transpose(3, 0, 2, 1, 4)
        .reshape(T, 128, DT * 128)).astype(BF_NP)
    b1t = np.ascontiguousarray(
        b1.reshape(E, HT, 128).transpose(2, 1, 0).reshape(128, T).astype(f))
    # stage-2 stationaries: expert e -> col group g=e%4, row k=e//4
    w2c = np.zeros((128, T, 32), dtype=f)
    for t in range(T):
        ht, e = divmod(t, E)
        k, g = divmod(e, 4)
        w2c[:, t, k] = W2[e, ht * 128:(ht + 1) * 128]
    w2c = w2c.reshape(128, T * 32).astype(BF_NP)
    # gating stationary: col 32g+k = Wg[:, 4k+g], rest zero
    wgp4 = np.zeros((DT, 128, 128), dtype=f)
    bg4 = np.full((128, 1), -30.0, dtype=f)
    ob2 = np.zeros((128, 2), dtype=f)
    ob2[:, 0] = 1.0
    for e in range(E):
        k, g = divmod(e, 4)
        wgp4[:, :, 32 * g + k] = Wg[:, e].reshape(DT, 128)
        bg4[32 * g + k, 0] = bg[e]
        ob2[32 * g + k, 1] = b2[e]
    wgp4 = np.ascontiguousarray(
        wgp4.transpose(1, 0, 2).reshape(128, DT * 128)).astype(BF_NP)
    ob2 = ob2.astype(BF_NP)
    shared = {"w1p": w1p, "b1t": b1t, "w2c": w2c, "wgp4": wgp4,
              "bg4": bg4, "ob2": ob2}
    xT = np.ascontiguousarray(np.asarray(x, dtype=f).T)  # [D, B]
    xts = []
    for c in range(N_CORES):
        xc = xT[:, c * BS:(c + 1) * BS]  # [D, BS]
        xc = np.ascontiguousarray(
            xc.reshape(DT, 128, BS).transpose(1, 0, 2).reshape(128, DT * BS))
        xts.append(xc.astype(BF_NP))
    return shared, xts


def run(inputs, trace=False):
    nc = build_bass()
    shared, xts = prep_inputs(**inputs)
    in_maps = [dict(shared, xt=xts[c]) for c in range(N_CORES)]
    res = run_bass_kernel_spmd(
        nc, in_maps, core_ids=list(range(N_CORES)), trace=trace
    )
    # y dram is [128, NCH] with y[p, j] = out[j*128 + p]
    y = np.concatenate(
        [np.asarray(r["y"], dtype=np.float32).T.reshape(BS, 1)
         for r in res.results], axis=0)
    return y, res


def kernel(**inputs):
    y, _ = run(inputs, trace=False)
    return y


if __name__ == "__main__":
    rng = np.random.default_rng(0)
    ins = {
        "x": rng.standard_normal((B, D), dtype=np.float32),
        "W1": rng.standard_normal((E, D, H), dtype=np.float32) / 32,
        "b1": rng.standard_normal((E, H), dtype=np.float32) / 32,
        "W2": rng.standard_normal((E, H), dtype=np.float32) / 32,
        "b2": rng.standard_normal((E,), dtype=np.float32) / 32,
        "Wg": rng.standard_normal((D, E), dtype=np.float32) / 32,
        "bg": rng.standard_normal((E,), dtype=np.float32) / 32,
    }
    y = kernel(**ins)
    print("ok", y.shape, y.dtype)
